# revision 6
# baseline (speedup 1.0000x reference)
"""Trainium2 Bass kernel for nn_CellAnnotator (sparse local attention + MLP decoder).

Computation (per reference):
  w = sigmoid(x)                               # per-pixel 8x8 local window weights
  x0[i,j,c] = sum_{di,dj} xpad[i+di, j+dj, c] * w[i, j, 8*di+dj]   (SAME pad (3,4))
  4x [ y = relu(x0 @ W_l + b_l); x0 = LN(y)*gamma_l + beta_l ]
  out = x0 @ w_out + b_out
Returns (x0, out).

Sharding: H split across 8 cores (96 rows each + 7 halo rows from host-padded input).
Attention: per-pixel scalar_tensor_tensor FMAs on VectorE (pixels on partitions,
channels on free dim); per source row, 8 column-shifted views are loaded via one
DMA each (contiguous 2KB runs). MLP: TensorE matmuls channels-on-partition with
LayerNorm handled via folded weights + deferred per-pixel scale (inv) / shift (mu)
applied with broadcast matmuls on the PE.
"""

import numpy as np

H = 768
W = 768
C = 64
NL = 4
LN_EPS = 1e-6
NCORES = 8
RPC = H // NCORES            # output rows per core: 96
PR = RPC + 7                 # padded rows per core: 103
WPAD = W + 7                 # padded width: 775
HALF = W // 2                # 384 cols per half-pass
NT = HALF // 128             # 3 col-tiles of 128 per half
GR = 4                       # rows per MLP group
FG = GR * HALF               # 1536 pixels per group
CH = 512                     # matmul chunk (one PSUM bank)
NCH = FG // CH               # 3 chunks

_PROG = None


def _build_program():
    import concourse.bass as bass
    import concourse.bacc as bacc
    import concourse.tile as tile
    from concourse import mybir
    from concourse.masks import make_identity

    f32 = mybir.dt.float32
    AL = mybir.AluOpType
    AF = mybir.ActivationFunctionType

    nc = bacc.Bacc("TRN2", target_bir_lowering=False)

    xin = nc.dram_tensor("xin", [PR, WPAD, C], f32, kind="ExternalInput")
    weff = nc.dram_tensor("weff", [NL, C, C], f32, kind="ExternalInput")
    bact = nc.dram_tensor("bact", [NL, C], f32, kind="ExternalInput")
    uneg = nc.dram_tensor("uneg", [NL, C], f32, kind="ExternalInput")
    g3 = nc.dram_tensor("g3", [C], f32, kind="ExternalInput")
    g3n = nc.dram_tensor("g3n", [C], f32, kind="ExternalInput")
    b3 = nc.dram_tensor("b3", [C], f32, kind="ExternalInput")
    wo = nc.dram_tensor("wo", [C, 1], f32, kind="ExternalInput")
    bo = nc.dram_tensor("bo", [1, 1], f32, kind="ExternalInput")
    x0o = nc.dram_tensor("x0o", [RPC, W, C], f32, kind="ExternalOutput")
    outo = nc.dram_tensor("outo", [RPC, W], f32, kind="ExternalOutput")

    with tile.TileContext(nc) as tc:
        with (
            tc.tile_pool(name="singles", bufs=1) as singles,
            tc.tile_pool(name="xv", bufs=9) as xv_pool,
            tc.tile_pool(name="w2", bufs=2) as w2_pool,
            tc.tile_pool(name="acc", bufs=6) as acc_pool,
            tc.tile_pool(name="mlp", bufs=2) as mlp_pool,
            tc.tile_pool(name="x0t", bufs=2) as x0t_pool,
            tc.tile_pool(name="small", bufs=3) as small_pool,
            tc.tile_pool(name="orow", bufs=2) as orow_pool,
            tc.tile_pool(name="stats", bufs=2) as stats_pool,
            tc.tile_pool(name="stage", bufs=2) as stage_pool,
            tc.tile_pool(name="pst", bufs=1, space="PSUM") as pst_pool,
            tc.tile_pool(name="pmmP", bufs=2, space="PSUM") as pmmP_pool,
            tc.tile_pool(name="pmmG", bufs=1, space="PSUM") as pmmG_pool,
            tc.tile_pool(name="pmmH", bufs=1, space="PSUM") as pmmH_pool,
            tc.tile_pool(name="pss", bufs=2, space="PSUM") as pss_pool,
        ):
            # ---- constants / weights in SBUF ----
            weff_sb = singles.tile([C, NL, C], f32)
            nc.sync.dma_start(out=weff_sb[:, :, :],
                              in_=weff.ap().rearrange("l ci co -> ci l co"))
            bact_sb = singles.tile([C, NL], f32)
            nc.sync.dma_start(out=bact_sb[:, :],
                              in_=bact.ap().rearrange("l c -> c l"))
            uneg_sb = singles.tile([1, NL * C], f32)
            nc.sync.dma_start(out=uneg_sb[:, :],
                              in_=uneg.ap().rearrange("l c -> (l c)"))
            g3_sb = singles.tile([1, C], f32)
            nc.sync.dma_start(out=g3_sb[:, :], in_=g3.ap())
            g3n_sb = singles.tile([1, C], f32)
            nc.sync.dma_start(out=g3n_sb[:, :], in_=g3n.ap())
            b3_sb = singles.tile([1, C], f32)
            nc.sync.dma_start(out=b3_sb[:, :], in_=b3.ap())
            wo_sb = singles.tile([C, 1], f32)
            nc.sync.dma_start(out=wo_sb[:, :], in_=wo.ap())
            bo_sb = singles.tile([1, 1], f32)
            nc.sync.dma_start(out=bo_sb[:, :], in_=bo.ap())
            o64 = singles.tile([C, 1], f32)
            nc.vector.memset(o64, 1.0 / C)
            ones1 = singles.tile([1, C], f32)
            nc.vector.memset(ones1, 1.0)
            ones_row = singles.tile([1, FG], f32)
            nc.vector.memset(ones_row, 1.0)
            id128 = singles.tile([128, 128], f32)
            make_identity(nc, id128)

            for h in range(2):
                col0 = HALF * h  # padded-col base of this half's views
                xv = {}

                def load_row(r):
                    t = xv_pool.tile([128, NT, 8 * C], f32)
                    src = bass.AP(
                        tensor=xin,
                        offset=(r * WPAD + col0) * C,
                        ap=[[C, 128], [128 * C, NT], [1, 8 * C]],
                    )
                    nc.sync.dma_start(out=t[:, :, :], in_=src)
                    xv[r] = t

                for r in range(8):
                    load_row(r)

                accs = {}
                for i in range(RPC):
                    if i + 8 <= PR - 1:
                        load_row(i + 8)
                    # w2 = sigmoid(x at the output pixels) = dj=3 slice of row i+3
                    w2 = w2_pool.tile([128, NT, C], f32)
                    nc.scalar.activation(
                        out=w2[:, :, :],
                        in_=xv[i + 3][:, :, 3 * C:4 * C],
                        func=AF.Sigmoid,
                    )
                    acc = acc_pool.tile([128, NT, C], f32)
                    for t in range(NT):
                        a_sl = acc[:, t, :]
                        for k in range(64):
                            di, dj = k // 8, k % 8
                            x_sl = xv[i + di][:, t, dj * C:(dj + 1) * C]
                            w_sl = w2[:, t, k:k + 1]
                            if k == 0:
                                nc.vector.tensor_scalar_mul(
                                    out=a_sl, in0=x_sl, scalar1=w_sl)
                            else:
                                nc.vector.scalar_tensor_tensor(
                                    out=a_sl, in0=x_sl, scalar=w_sl, in1=a_sl,
                                    op0=AL.mult, op1=AL.add)
                    accs[i] = acc
                    if i >= 7:
                        xv.pop(i - 7, None)

                    if i % GR == GR - 1:
                        i0 = i - (GR - 1)
                        # ---- transpose-in: acc tiles -> R0 [64, FG] ----
                        R = mlp_pool.tile([C, FG], f32)
                        for g in range(GR):
                            for t in range(NT):
                                slot = g * NT + t
                                ps = pst_pool.tile([128, 128], f32, tag="ps")
                                nc.tensor.transpose(
                                    ps[0:C, :], accs[i0 + g][:, t, :], id128[:, :])
                                nc.scalar.activation(
                                    out=R[:, slot * 128:(slot + 1) * 128],
                                    in_=ps[0:C, :], func=AF.Copy)
                        for g in range(GR):
                            accs.pop(i0 + g, None)

                        stats = {}
                        for l in range(NL):
                            Rn = mlp_pool.tile([C, FG], f32)
                            for ci in range(NCH):
                                sl = slice(ci * CH, (ci + 1) * CH)
                                P_ps = pmmP_pool.tile([C, CH], f32, tag="P")
                                nc.tensor.matmul(
                                    P_ps[:, :], weff_sb[:, l, :], R[:, sl],
                                    start=True, stop=True)
                                if l == 0:
                                    nc.scalar.activation(
                                        out=Rn[:, sl], in_=P_ps[:, :],
                                        func=AF.Relu,
                                        bias=bact_sb[:, l:l + 1], scale=1.0)
                                else:
                                    invp, nmip = stats[l - 1]
                                    G_ps = pmmG_pool.tile([C, CH], f32, tag="G")
                                    nc.tensor.matmul(
                                        G_ps[:, :], ones1[:, :],
                                        invp[:, sl], start=True, stop=True)
                                    H_ps = pmmH_pool.tile([C, CH], f32, tag="H")
                                    nc.tensor.matmul(
                                        H_ps[:, :],
                                        uneg_sb[:, l * C:(l + 1) * C],
                                        nmip[:, sl], start=True, stop=True)
                                    g_sb = small_pool.tile([C, CH], f32, tag="g")
                                    nc.scalar.activation(
                                        out=g_sb[:, :], in_=G_ps[:, :],
                                        func=AF.Copy)
                                    q_sb = small_pool.tile([C, CH], f32, tag="q")
                                    nc.vector.tensor_tensor(
                                        out=q_sb[:, :], in0=P_ps[:, :],
                                        in1=g_sb[:, :], op=AL.mult)
                                    z_sb = small_pool.tile([C, CH], f32, tag="z")
                                    nc.vector.tensor_tensor(
                                        out=z_sb[:, :], in0=q_sb[:, :],
                                        in1=H_ps[:, :], op=AL.add)
                                    nc.scalar.activation(
                                        out=Rn[:, sl], in_=z_sb[:, :],
                                        func=AF.Relu,
                                        bias=bact_sb[:, l:l + 1], scale=1.0)
                            # ---- stats of y_l over channels (per pixel) ----
                            mu_t = stats_pool.tile([1, FG], f32, tag="mu")
                            aa_t = stats_pool.tile([1, FG], f32, tag="aa")
                            for ci in range(NCH):
                                sl = slice(ci * CH, (ci + 1) * CH)
                                s1 = pss_pool.tile([1, CH], f32, tag="sps")
                                nc.tensor.matmul(s1[:, :], o64[:, :], Rn[:, sl],
                                                 start=True, stop=True)
                                nc.scalar.activation(out=mu_t[:, sl], in_=s1[:, :],
                                                     func=AF.Copy)
                                ysq = small_pool.tile([C, CH], f32, tag="ysq")
                                nc.scalar.activation(out=ysq[:, :], in_=Rn[:, sl],
                                                     func=AF.Square)
                                s2 = pss_pool.tile([1, CH], f32, tag="sps")
                                nc.tensor.matmul(s2[:, :], o64[:, :], ysq[:, :],
                                                 start=True, stop=True)
                                nc.scalar.activation(out=aa_t[:, sl], in_=s2[:, :],
                                                     func=AF.Copy)
                            qq_t = stats_pool.tile([1, FG], f32, tag="qq")
                            nc.scalar.activation(out=qq_t[:, :], in_=mu_t[:, :],
                                                 func=AF.Square)
                            nc.vector.scalar_tensor_tensor(
                                out=aa_t[:, :], in0=aa_t[:, :], scalar=LN_EPS,
                                in1=qq_t[:, :], op0=AL.add, op1=AL.subtract)
                            nc.scalar.activation(out=qq_t[:, :], in_=aa_t[:, :],
                                                 func=AF.Sqrt)
                            inv_t = aa_t
                            nc.vector.reciprocal(out=inv_t[:, :], in_=qq_t[:, :])
                            nmi_t = mu_t
                            nc.vector.tensor_tensor(
                                out=nmi_t[:, :], in0=mu_t[:, :], in1=inv_t[:, :],
                                op=AL.mult)
                            stats[l] = (inv_t, nmi_t)
                            R = Rn

                        # ---- x0 = y3*G3 + H3 ; out = x0 @ wo + bo ----
                        inv3, nmi3 = stats[NL - 1]
                        x0T = x0t_pool.tile([C, FG], f32)
                        outrow = orow_pool.tile([1, FG], f32, tag="outrow")
                        for ci in range(NCH):
                            sl = slice(ci * CH, (ci + 1) * CH)
                            G_ps = pmmG_pool.tile([C, CH], f32, tag="G")
                            nc.tensor.matmul(G_ps[:, :], g3_sb[:, :], inv3[:, sl],
                                             start=True, stop=True)
                            H_ps = pmmH_pool.tile([C, CH], f32, tag="H")
                            nc.tensor.matmul(H_ps[:, :], g3n_sb[:, :], nmi3[:, sl],
                                             start=True, stop=False)
                            nc.tensor.matmul(H_ps[:, :], b3_sb[:, :], ones_row[:, sl],
                                             start=False, stop=True)
                            q_sb = small_pool.tile([C, CH], f32, tag="q")
                            nc.vector.tensor_tensor(out=q_sb[:, :], in0=R[:, sl],
                                                    in1=G_ps[:, :], op=AL.mult)
                            nc.vector.tensor_tensor(out=x0T[:, sl], in0=q_sb[:, :],
                                                    in1=H_ps[:, :], op=AL.add)
                            O_ps = pss_pool.tile([1, CH], f32, tag="sps")
                            nc.tensor.matmul(O_ps[:, :], wo_sb[:, :], x0T[:, sl],
                                             start=True, stop=True)
                            nc.scalar.activation(out=outrow[:, sl], in_=O_ps[:, :],
                                                 func=AF.Identity,
                                                 bias=bo_sb[:, :], scale=1.0)

                        # ---- transpose-out + stores ----
                        x0st = stage_pool.tile([128, GR * NT, C], f32)
                        for slot in range(GR * NT):
                            ps = pst_pool.tile([128, 128], f32, tag="ps")
                            nc.tensor.transpose(
                                ps[:, 0:C], x0T[:, slot * 128:(slot + 1) * 128],
                                id128[0:C, 0:C])
                            nc.scalar.activation(out=x0st[:, slot, :], in_=ps[:, 0:C],
                                                 func=AF.Copy)
                        for g in range(GR):
                            dst = bass.AP(
                                tensor=x0o,
                                offset=((i0 + g) * W + HALF * h) * C,
                                ap=[[C, 128], [128 * C, NT], [1, C]],
                            )
                            nc.sync.dma_start(
                                out=dst, in_=x0st[:, g * NT:(g + 1) * NT, :])
                        odst = bass.AP(
                            tensor=outo,
                            offset=i0 * W + HALF * h,
                            ap=[[W, GR], [128, NT], [1, 128]],
                        )
                        nc.sync.dma_start(out=odst, in_=outrow[:, :])

    nc.compile()
    return nc


def _get_program():
    global _PROG
    if _PROG is None:
        _PROG = _build_program()
    return _PROG


def kernel(x, Ws, bs, ln_scale, ln_bias, w_out, b_out):
    from concourse.bass_utils import run_bass_kernel_spmd

    x = np.asarray(x, np.float32)
    Ws = np.asarray(Ws, np.float32)
    bs = np.asarray(bs, np.float32)
    ln_scale = np.asarray(ln_scale, np.float32)
    ln_bias = np.asarray(ln_bias, np.float32)
    w_out = np.asarray(w_out, np.float32)
    b_out = np.asarray(b_out, np.float32)

    # host-side folding of LayerNorm affine into the next layer's weights
    W_eff = np.empty_like(Ws)
    bact = np.empty_like(bs)
    uneg = np.zeros_like(bs)
    W_eff[0] = Ws[0]
    bact[0] = bs[0]
    for l in range(1, NL):
        W_eff[l] = ln_scale[l - 1][:, None] * Ws[l]
        bact[l] = ln_bias[l - 1] @ Ws[l] + bs[l]
        uneg[l] = -W_eff[l].sum(axis=0)
    g3 = ln_scale[NL - 1].copy()
    g3nv = -g3
    b3v = ln_bias[NL - 1].copy()
    bo = np.asarray(b_out, np.float32).reshape(1, 1)

    xpad = np.pad(x, ((3, 4), (3, 4), (0, 0))).astype(np.float32)

    nc = _get_program()
    in_maps = []
    for m in range(NCORES):
        in_maps.append({
            "xin": np.ascontiguousarray(xpad[RPC * m: RPC * m + PR]),
            "weff": W_eff, "bact": bact, "uneg": uneg,
            "g3": g3, "g3n": g3nv, "b3": b3v,
            "wo": w_out.reshape(C, 1), "bo": bo,
        })
    res = run_bass_kernel_spmd(nc, in_maps, core_ids=list(range(NCORES)))
    global LAST_RESULTS
    LAST_RESULTS = res
    x0 = np.concatenate([res.results[m]["x0o"] for m in range(NCORES)], axis=0)
    out = np.concatenate([res.results[m]["outo"] for m in range(NCORES)], axis=0)
    return (x0, out[:, :, None])


LAST_RESULTS = None
